# revision 6
# baseline (speedup 1.0000x reference)
"""Depth-wise attention over block outputs (AttentionResidual) on 8 trn2 cores.

Computation (reference):
    q' = proj[min(block_idx, maxT-1)] * norm_scale
    rms[t,r]   = sqrt(mean_d(e[t,r,:]^2) + 1e-5)
    logit[t,r] = (q' . e[t,r,:]) / rms[t,r]      (masked -1e9 for t >= n_active)
    w = softmax_t(logit);  out[r,:] = sum_t w[t,r] * e[t,r,:]

Masked entries (t >= n_active) get softmax weight exactly 0.0 in fp32, so
only the first n_active depth slices are ever read (12/16 of the input).

Sharding: flattened B*S rows split evenly across the 8 cores (data
parallel); q' and the identity are replicated. No cross-core reduction.

Design (measured on trn2 via axon):
  - entries are cast f32 -> fp16 during the SWDGE load. fp16 (10-bit
    mantissa) keeps the softmax-amplified logit noise ~8x below bf16;
    end-to-end max-rel error ~1.1e-2 vs the 2e-2 gate. Halves SBUF
    footprint and runs the weighted-sum matmuls at 1 cyc/row (4x vs f32).
  - per 128-row tile: one SWDGE DMA loads e[128, T, D] fp16 (~6.3 MiB
    HBM read); ACT does the 12 Square+accum -> ssq with the dummy
    elementwise output aimed at PSUM (SBUF writes would contend with the
    DMA write side); DVE does the 12 q'.e_t reductions the same way, plus
    rsqrt (integer-seed Newton), softmax stats, and the diag(w_t)
    construction; TensorE accumulates the 24 fp16 diag(w_t) @ e_t
    matmuls into PSUM; ACT evacuates PSUM -> fp16 SBUF one tile late
    (software-pipelined so ACT never stalls on the matmul chain); HWDGE
    stores fp16 output (widened to f32 on the host).
  - measured ~328 us/iter per core vs a ~311 us pure-DMA floor
    (96 MiB read + 4 MiB write at ~313 GB/s/core effective), ~1.9x
    faster than the previous f32 kernel (613 us).
"""

import sys

import numpy as np

sys.path.insert(0, "/opt/trn_rl_repo")

import concourse.bacc as bacc  # noqa: E402
import concourse.bass as bass  # noqa: E402
import concourse.tile as tile  # noqa: E402
from concourse import mybir  # noqa: E402
from concourse.bass_utils import run_bass_kernel_spmd  # noqa: E402

N_CORES = 8
P = 128
NORM_EPS = 1e-5

_kernel_cache = {}


def _build_kernel(T_act, R, D, rep=1):
    """Per-core Bass program. T_act: active depth entries; R: rows per core;
    D: feature dim. rep: re-run the body rep times via a device loop
    (benchmarking aid; output is idempotent)."""
    f32 = mybir.dt.float32
    f16 = mybir.dt.float16
    AF = mybir.ActivationFunctionType
    OP = mybir.AluOpType
    assert R % P == 0 and D % 512 == 0
    n_tiles = R // P
    nh = D // 512  # matmul halves (PSUM bank = 512 fp32)

    nc = bacc.Bacc()
    ent = nc.declare_dram_parameter("ent", [T_act, R, D], f32, isOutput=False)
    qv = nc.declare_dram_parameter("qv", [1, D], f16, isOutput=False)
    ident = nc.declare_dram_parameter("ident", [P, P], f16, isOutput=False)
    out = nc.declare_dram_parameter("out", [R, D], f16, isOutput=True)

    with tile.TileContext(nc) as tc:
        with (
            tc.tile_pool(name="singles", bufs=1) as singles,
            tc.tile_pool(name="ebuf", bufs=(7 if T_act <= 12 else 4)) as ebuf,
            tc.tile_pool(name="stats", bufs=2) as stats,
            tc.tile_pool(name="diag", bufs=3) as diagp,
            tc.tile_pool(name="outs", bufs=3) as outs,
            tc.tile_pool(name="psum", bufs=2, space="PSUM") as psump,
            tc.tile_pool(name="pscr", bufs=1, space="PSUM") as pscr,
        ):
            qb = singles.tile([P, D], f16)
            nc.gpsimd.dma_start(out=qb, in_=qv[:, :].to_broadcast((P, D)))
            id_t = singles.tile([P, P], f16)
            nc.sync.dma_start(out=id_t, in_=ident[:, :])

            import contextlib

            loop_ctx = tc.For_i(0, rep, 1) if rep > 1 else contextlib.nullcontext()
            with loop_ctx:
                _loop_body(nc, T_act, D, n_tiles, nh, ent, out, qb, id_t,
                           ebuf, stats, diagp, outs, psump, pscr)

    nc.finalize()
    return nc


def _loop_body(nc, T_act, D, n_tiles, nh, ent, out, qb, id_t,
               ebuf, stats, diagp, outs, psump, pscr):
    f32 = mybir.dt.float32
    f16 = mybir.dt.float16
    u32 = mybir.dt.uint32
    AF = mybir.ActivationFunctionType
    OP = mybir.AluOpType
    pending = None  # (po, r0) awaiting PSUM evac + store, pipelined one tile
    for i in range(n_tiles):
        r0 = i * P
        e = ebuf.tile([P, T_act, D], f16, tag="e")
        # one SWDGE cast-DMA per t-slice: each reads a contiguous 512 KiB
        # HBM block (sequential access is ~6% faster than the single
        # t-interleaved DMA whose reads jump 8 MiB every 4 KiB)
        for t in range(T_act):
            nc.gpsimd.dma_start(
                out=e[:, t, :],
                in_=ent[t, r0 : r0 + P, :],
            )

        ssq = stats.tile([P, T_act], f32)
        qd = stats.tile([P, T_act], f32)
        sqa_scr = pscr.tile([P, D], f32, tag="sqa")  # ACT dummy out (PSUM)
        qd_scr = pscr.tile([P, D], f32, tag="qds")   # DVE dummy out (PSUM)
        for t in range(T_act):
            nc.scalar.activation(
                out=sqa_scr,
                in_=e[:, t, :],
                func=AF.Square,
                accum_out=ssq[:, t : t + 1],
            )
            nc.vector.scalar_tensor_tensor(
                out=qd_scr,
                in0=e[:, t, :],
                scalar=0.0,
                in1=qb,
                op0=OP.bypass,
                op1=OP.mult,
                accum_out=qd[:, t : t + 1],
            )

        # ms = ssq/D + eps; rinv = rsqrt(ms) via integer-seed Newton
        # (keeps ACT's LUT set fixed: only Square/Exp/Copy used).
        ms = stats.tile([P, T_act], f32)
        nc.vector.tensor_scalar(
            out=ms, in0=ssq, scalar1=1.0 / D, scalar2=float(NORM_EPS),
            op0=OP.mult, op1=OP.add,
        )
        sh = stats.tile([P, T_act], u32)
        nc.vector.tensor_scalar(
            out=sh, in0=ms[:].bitcast(u32), scalar1=1, scalar2=None,
            op0=OP.logical_shift_right,
        )
        shf = stats.tile([P, T_act], f32)
        nc.vector.tensor_copy(shf, sh)
        nc.vector.tensor_scalar(
            out=shf, in0=shf, scalar1=-1.0, scalar2=float(0x5F3759DF),
            op0=OP.mult, op1=OP.add,
        )
        yb = stats.tile([P, T_act], u32)
        nc.vector.tensor_copy(yb, shf)
        rinv = yb[:].bitcast(f32)
        nwt = stats.tile([P, T_act], f32)
        for _ in range(2):
            nc.vector.tensor_mul(nwt, rinv, rinv)
            nc.vector.tensor_mul(nwt, nwt, ms)
            nc.vector.tensor_scalar(
                out=nwt, in0=nwt, scalar1=-0.5, scalar2=1.5,
                op0=OP.mult, op1=OP.add,
            )
            nc.vector.tensor_mul(rinv, rinv, nwt)

        lg = stats.tile([P, T_act], f32)
        nc.vector.tensor_mul(lg, qd, rinv)

        # softmax over the free (t) axis; 1/sum folded into the diagonals
        mx = stats.tile([P, 1], f32)
        nc.vector.tensor_reduce(
            out=mx, in_=lg, axis=mybir.AxisListType.X, op=OP.max
        )
        negm = stats.tile([P, 1], f32)
        nc.vector.tensor_scalar_mul(negm, mx, -1.0)
        ex = stats.tile([P, T_act], f32)
        sume = stats.tile([P, 1], f32)
        nc.scalar.activation(
            out=ex, in_=lg, func=AF.Exp, bias=negm, accum_out=sume
        )
        rsum = stats.tile([P, 1], f32)
        nc.vector.reciprocal(rsum, sume)

        # w = ex * rsum, then dg[p, t, c] = id[p, c] * w[p, t] in ONE
        # DVE op via stride-0 broadcast APs (fewer DVE dispatches than 12
        # per-t tensor_scalar ops; measured -15 us/iter)
        w = stats.tile([P, T_act], f32)
        nc.vector.tensor_scalar_mul(w, ex, rsum[:, 0:1])
        dg_all = diagp.tile([P, T_act, P], f16, tag="dg")
        ida = id_t[:, :]
        wa = w[:, 0:T_act]
        idb = bass.AP(tensor=ida.tensor, offset=ida.offset,
                      ap=[ida.ap[0], [0, T_act], ida.ap[1]])
        wb = bass.AP(tensor=wa.tensor, offset=wa.offset,
                     ap=[wa.ap[0], wa.ap[1], [0, P]])
        nc.vector.tensor_tensor(out=dg_all, in0=idb, in1=wb, op=OP.mult)

        po = psump.tile([P, D], f32)
        for h in range(nh):
            cs = slice(h * 512, (h + 1) * 512)
            for t in range(T_act):
                nc.tensor.matmul(
                    po[:, cs],
                    lhsT=dg_all[:, t, :],
                    rhs=e[:, t, cs],
                    start=(t == 0),
                    stop=(t == T_act - 1),
                )

        if pending is not None:
            _evac(nc, pending[0], pending[1], outs, out)
        pending = (po, r0)

    if pending is not None:
        _evac(nc, pending[0], pending[1], outs, out)


def _evac(nc, po, r0, outs, out):
    """PSUM -> fp16 SBUF evac split between ACT and DVE, then HWDGE store.
    Runs one tile late so neither engine stalls on the matmul chain."""
    Dh = po.shape[1] // 2
    ob = outs.tile([P, po.shape[1]], mybir.dt.float16)
    nc.scalar.copy(out=ob[:, 0:Dh], in_=po[:, 0:Dh])
    nc.vector.tensor_copy(ob[:, Dh:], po[:, Dh:])
    nc.sync.dma_start(out=out[r0 : r0 + P, :], in_=ob)


def _get_kernel(T_act, R, D):
    key = (T_act, R, D)
    if key not in _kernel_cache:
        _kernel_cache[key] = _build_kernel(T_act, R, D)
    return _kernel_cache[key]


def kernel(entries, proj, norm_scale, n_active, block_idx):
    entries = np.asarray(entries)
    proj = np.asarray(proj, dtype=np.float32)
    norm_scale = np.asarray(norm_scale, dtype=np.float32)
    if entries.dtype != np.float32:
        entries = entries.astype(np.float32)
    maxT, B, S, D = entries.shape
    na = int(np.asarray(n_active))
    bi = int(np.asarray(block_idx))

    if na <= 0:
        # everything masked -> softmax of equal (-1e9) logits = uniform mean
        T_act = maxT
        qprime = np.zeros((D,), dtype=np.float32)
    else:
        T_act = min(na, maxT)
        qprime = (proj[min(bi, maxT - 1)] * norm_scale).astype(np.float32)

    rows = B * S
    assert rows % (N_CORES * P) == 0, f"rows={rows} not divisible by {N_CORES * P}"
    R = rows // N_CORES

    ent_flat = entries[:T_act].reshape(T_act, rows, D)
    ident = np.eye(P, dtype=np.float16)
    qv = qprime.reshape(1, D).astype(np.float16)

    nc = _get_kernel(T_act, R, D)

    in_maps = []
    for c in range(N_CORES):
        in_maps.append({
            "ent": np.ascontiguousarray(ent_flat[:, c * R : (c + 1) * R, :]),
            "qv": qv,
            "ident": ident,
        })

    res = run_bass_kernel_spmd(nc, in_maps, list(range(N_CORES)))
    global _last_results
    _last_results = res
    parts = [res.results[c]["out"] for c in range(N_CORES)]
    return np.concatenate(parts, axis=0).reshape(B, S, D).astype(np.float32)


_last_results = None
